# revision 36
# baseline (speedup 1.0000x reference)
"""Trainium2 Bass kernel for the attention-scoring MLP (nn_Attn):

    enc = encoder_outputs.transpose(1,0,2)          # [B,S,Hin]
    a1  = tanh(enc @ W1_enc.T + hidden @ W1_hid.T + b1)
    s   = a1 @ W2[0] (+ b2 -- dropped: softmax shift-invariant)
    s   = where(mask, -inf, s)
    out = softmax(s, axis=-1)[:, None, :]           # [B,1,S]

Sharding: data-parallel over batch B=32 across 8 NeuronCores (4 rows
each), weights replicated, no collectives.

Mask packing: masked positions get score -inf and contribute nothing to
the softmax, so the kernel only computes the ~50% unmasked columns.
kernel() gathers each row's unmasked enc columns into a packed layout of
CB columns per row (CB = 576 covers the binomial spread; zero columns +
-1e30 mask data pad the remainder, so the instruction stream is
SPMD-uniform across cores -- only DMA'd data differs). The host scatters
the packed attention weights back to full [B,1,S] (pure layout, no
arithmetic).

Per core the main matmul is computed transposed -- a1T[h, s] =
W1_encT.T @ encT -- so the (b1 + hidden@W1_hid.T) term rides the
ScalarEngine's per-partition bias port of the tanh activation. Matmuls
run in fp8 (e4m3) with MatmulPerfMode.DoubleRow: each matmul contracts a
PAIR of 128-row k-tiles (lhsT/rhs laid out [128, 2, n]), the PE
streaming 2 fp8 rows/cycle. Weights are pre-scaled by 32 host-side so
their +-1/sqrt(2048) range sits in fp8's normal range; the 1/32 is
folded into the tanh activation scale. Accumulation is fp32 in PSUM.
Each tile's enc arrives in ONE DMA instruction (descriptor generation at
~600ns/instruction was throttling the pipeline when split per k-tile).

The w2 score contraction runs as an in-place DVE accumulate chain over
ht (acc = w2[:,ht]*th_ht + acc, per-partition scalars) plus a single
bf16 ones-matmul to reduce partitions; the final tile instead issues M=1
score matmuls on the PE so the tail never waits on the DVE chain.
"""

import numpy as np
import ml_dtypes

import concourse.bass as bass
import concourse.tile as tile
from concourse import bacc, mybir
from concourse.bass import ds, ts
from concourse.bass_utils import run_bass_kernel_spmd
from concourse.masks import make_identity

N_CORES = 8
B, S, HIN, H = 32, 1024, 1024, 1024
BL = B // N_CORES          # local batch rows per core
P = 128                    # partitions
IT = HIN // P              # contraction k-tiles
KP = IT // 2               # k-tile pairs (DoubleRow)
HT = H // P                # output-feature tiles
NT = 512                   # moving-dim tile (s columns per matmul)
BP = 16                    # padded batch rows (dual-fp8 ldweights step%16)
WS = 32.0                  # host-side weight scale (undone in act scale)
F32 = mybir.dt.float32
BF16 = mybir.dt.bfloat16
FP8 = mybir.dt.float8e4
AF = mybir.ActivationFunctionType
DR = mybir.MatmulPerfMode.DoubleRow
F8 = ml_dtypes.float8_e4m3

_cached = {}
LAST_RESULT = None  # BassKernelResults of the most recent run (for test harness)


def _layout(CB):
    """Static tile/segment layout for packed width CB (multiple of 64)."""
    TCOLS = BL * CB
    tiles = []  # (col0, nt, [(off, ln, b), ...])
    col = 0
    while col < TCOLS:
        nt = min(NT, TCOLS - col)
        segs = []
        o = col
        while o < col + nt:
            b = o // CB
            end = min((b + 1) * CB, col + nt)
            segs.append((o - col, end - o, b))
            o = end
        tiles.append((col, nt, segs))
        col += nt
    slot_of = {}
    b_slots = {b: [] for b in range(BL)}
    s = 0
    for ti, (_, _, segs) in enumerate(tiles):
        for si, (_, _, b) in enumerate(segs):
            slot_of[(ti, si)] = s
            b_slots[b].append(s)
            s += 1
    return TCOLS, tiles, slot_of, b_slots, s


def _build(CB):
    if CB in _cached:
        return _cached[CB]
    TCOLS, tiles, slot_of, b_slots, NSEG = _layout(CB)
    NTI = len(tiles)

    nc = bacc.Bacc("TRN2", target_bir_lowering=False, debug=False,
                   num_devices=N_CORES)

    # packed encT: [it, p, col] fp8 (k = it*128+p; col = b*CB + packed s)
    enc_ext = nc.dram_tensor("enc", [IT, P, TCOLS], FP8, kind="ExternalInput").ap()
    # hiddenT packed+padded [p, it, bp] fp8 (k = it*128+p; bp 0..3 real)
    hidt_ext = nc.dram_tensor("hiddent", [P, IT, BP], FP8, kind="ExternalInput").ap()
    mneg_ext = nc.dram_tensor("maskneg", [TCOLS], F32, kind="ExternalInput").ap()
    # W1_enc.T packed per ht: [ht, p, it, m] = 32*W1[ht*128+m, it*128+p]
    w1e_ext = nc.dram_tensor("w1e", [HT, P, IT, P], FP8, kind="ExternalInput").ap()
    # W1_hid.T packed: [p, it, h]
    w1h_ext = nc.dram_tensor("w1h", [P, IT, H], FP8, kind="ExternalInput").ap()
    b1_ext = nc.dram_tensor("b1", [H], F32, kind="ExternalInput").ap()
    # W2 packed [p, ht] f32 (h = ht*128+p)
    w2_ext = nc.dram_tensor("w2", [P, HT], F32, kind="ExternalInput").ap()
    out_ext = nc.dram_tensor("out", [TCOLS], F32, kind="ExternalOutput").ap()

    encR = enc_ext.rearrange("it p c -> p it c")

    with tile.TileContext(nc) as tc:
        with (
            tc.tile_pool(name="consts", bufs=1) as consts,
            tc.tile_pool(name="encp", bufs=3) as encp,
            tc.tile_pool(name="thp", bufs=2) as thp,
            tc.tile_pool(name="accp", bufs=2) as accp,
            tc.tile_pool(name="pap", bufs=1, space="PSUM") as pap,
            tc.tile_pool(name="pscp", bufs=2, space="PSUM") as pscp,
            tc.tile_pool(name="psA", bufs=1, space="PSUM") as psA,
            tc.tile_pool(name="psT", bufs=1, space="PSUM") as psTp,
        ):
            # ---- resident weights/constants. A single DMA instruction runs
            # on ONE queue at ~68 GB/s, so the startup-critical transfers are
            # split across several rings/queues to run on parallel DMA
            # engines, ordered by deadline (w1e0+enc0 gate tile0's first
            # group, w1h gates phase A after tile0's 4th group).
            w1e_t = []
            w = consts.tile([P, IT, P], FP8, tag="w1e0", name="w1e0")
            nc.sync.dma_start(w[:], w1e_ext[0])
            w1e_t.append(w)
            enc0_sb = encp.tile([P, IT, NT], FP8, tag="enc", name="enc")
            nt0 = tiles[0][1]
            nc.scalar.dma_start(enc0_sb[:, 0:2, 0:nt0], encR[:, 0:2, ds(0, nt0)])
            nc.gpsimd.dma_start(enc0_sb[:, 2:4, 0:nt0], encR[:, 2:4, ds(0, nt0)])
            nc.vector.dma_start(enc0_sb[:, 4:6, 0:nt0], encR[:, 4:6, ds(0, nt0)])
            nc.sync.dma_start(enc0_sb[:, 6:8, 0:nt0], encR[:, 6:8, ds(0, nt0)])
            hT_sb = consts.tile([P, IT, BP], FP8)
            nc.sync.dma_start(hT_sb[:], hidt_ext[:])
            # w1h in thirds-ish on three rings
            w1h_sb = consts.tile([P, IT, H], FP8)
            nc.scalar.dma_start(w1h_sb[:, 0:3, :], w1h_ext[:, 0:3, :])
            nc.gpsimd.dma_start(w1h_sb[:, 3:6, :], w1h_ext[:, 3:6, :])
            nc.vector.dma_start(w1h_sb[:, 6:8, :], w1h_ext[:, 6:8, :])
            for ht in range(1, HT):
                w = consts.tile([P, IT, P], FP8, tag=f"w1e{ht}", name=f"w1e{ht}")
                nc.sync.dma_start(w[:], w1e_ext[ht])
                w1e_t.append(w)
            b1T_sb = consts.tile([P, HT], F32)
            nc.sync.dma_start(b1T_sb[:], b1_ext.rearrange("(ht p) -> p ht", p=P))
            w2T_sb = consts.tile([P, HT], F32)
            nc.sync.dma_start(w2T_sb[:], w2_ext[:])
            mneg_sb = consts.tile([1, TCOLS], F32)
            nc.sync.dma_start(mneg_sb[:], mneg_ext[:])

            # ---- PE warmup: junk matmuls with no DMA deps so the HAM
            # clock-gate / p-state ramp toward full speed during the preamble.
            warm_sb = consts.tile([P, NT], BF16)
            nc.gpsimd.memset(warm_sb[:], 0.0)
            # 4 persistent PSUM accumulators, rotated manually (tile-release
            # bookkeeping at kernel end costs ~30ns/allocation across the
            # engines' semaphore queues, so allocate once).
            pa1_t = [pap.tile([P, NT], F32, tag=f"pa1_{i}", name=f"pa1_{i}")
                     for i in range(4)]
            for _ in range(4):
                nc.tensor.matmul(pa1_t[0][:], warm_sb[:, 0:P], warm_sb[:],
                                 start=True, stop=True)

            ident_sb = consts.tile([BL, BL], F32)
            make_identity(nc, ident_sb[:])
            ones_sb = consts.tile([P, 1], BF16)
            nc.gpsimd.memset(ones_sb[:], 1.0)
            w2b_sb = consts.tile([P, HT], BF16)
            nc.vector.tensor_copy(w2b_sb[:], w2T_sb[:])

            bias_sb = consts.tile([P, HT * BL], F32)   # [p, ht*BL+b]
            hterm_sb = consts.tile([BL, H], F32)
            scores_sb = consts.tile([1, TCOLS], F32)
            c40 = consts.tile([1, 1], F32)
            nc.gpsimd.memset(c40[:], -40.0)
            exps = consts.tile([1, TCOLS], F32)
            ssum = consts.tile([1, NSEG], F32)
            rcp = consts.tile([1, BL], F32)
            attn = consts.tile([1, TCOLS], F32)

            def phase_a():
                # h_term[b,h] = hidden @ W1_hid.T (x32 in fp8, DoubleRow with
                # the batch dim padded to 16 so the dual-fp8 ldweights pair
                # step is 16); bias = h_termT/32 + b1T.
                ptT = psTp.tile([P, HT * BL], F32)
                for g in range(2):
                    pht = psA.tile([BP, NT], F32, tag="pht", name="pht")
                    for k in range(KP):
                        nc.tensor.matmul(pht[:],
                                         hT_sb[:, ds(2 * k, 2), :],
                                         w1h_sb[:, ds(2 * k, 2), ds(g * NT, NT)],
                                         start=(k == 0), stop=(k == KP - 1),
                                         perf_mode=DR)
                    nc.scalar.mul(hterm_sb[:, ds(g * NT, NT)], pht[0:BL, :],
                                  1.0 / WS)
                for ht in range(HT):
                    nc.tensor.transpose(ptT[:, ts(ht, BL)],
                                        hterm_sb[:, ts(ht, P)], ident_sb[:])
                    nc.vector.tensor_scalar_add(bias_sb[:, ts(ht, BL)],
                                                ptT[:, ts(ht, BL)],
                                                b1T_sb[:, ds(ht, 1)])

            # ---- phase B ----
            def flush_scores(ti, acc_f, psc=None):
                c0, nt, segs = tiles[ti]
                if psc is None:
                    psc = pscp.tile([1, NT], F32, tag="psc", name="psc")
                    nc.tensor.matmul(psc[0:1, 0:nt], ones_sb[:],
                                     acc_f[:, 0:nt], start=True, stop=True)
                # scores += mask * -1e30 (also kills the packing pad columns)
                nc.vector.tensor_add(scores_sb[0:1, ds(c0, nt)],
                                     psc[0:1, 0:nt],
                                     mneg_sb[0:1, ds(c0, nt)])
                # |scores| <= ||W2||_1 <= 32: exp(s - 40) never overflows and
                # softmax is shift-invariant -- no max-reduce needed.
                for si, (off, ln, b) in enumerate(segs):
                    slot = slot_of[(ti, si)]
                    nc.scalar.activation(exps[0:1, ds(c0 + off, ln)],
                                         scores_sb[0:1, ds(c0 + off, ln)],
                                         AF.Exp, bias=c40[0:1, 0:1], scale=1.0)
                    nc.vector.reduce_sum(ssum[0:1, ds(slot, 1)],
                                         exps[0:1, ds(c0 + off, ln)],
                                         axis=mybir.AxisListType.X)
                    if slot == b_slots[b][-1]:
                        s0 = b_slots[b][0]
                        nsl = len(b_slots[b])
                        nc.vector.reduce_sum(rcp[0:1, ds(b, 1)],
                                             ssum[0:1, ds(s0, nsl)],
                                             axis=mybir.AxisListType.X)
                        nc.vector.reciprocal(rcp[0:1, ds(b, 1)],
                                             rcp[0:1, ds(b, 1)])
                        nc.vector.tensor_scalar_mul(attn[0:1, ds(b * CB, CB)],
                                                    exps[0:1, ds(b * CB, CB)],
                                                    rcp[0:1, ds(b, 1)])
                        nc.sync.dma_start(out_ext[ds(b * CB, CB)],
                                          attn[0:1, ds(b * CB, CB)])

            prev = None  # (tile idx, final acc tile) awaiting score flush
            for ti, (c0, nt, segs) in enumerate(tiles):
                last = ti == NTI - 1
                if ti == 0:
                    enc_sb = enc0_sb
                else:
                    # two half-tile DMAs on different rings -> two queues in
                    # parallel (~3.8us/tile), triggered ~2 tiles ahead of use.
                    enc_sb = encp.tile([P, IT, NT], FP8, tag="enc", name="enc")
                    ea, eb = ((nc.scalar, nc.gpsimd) if ti == 1
                              else (nc.sync, nc.vector))
                    ea.dma_start(enc_sb[:, 0:4, 0:nt], encR[:, 0:4, ds(c0, nt)])
                    eb.dma_start(enc_sb[:, 4:8, 0:nt], encR[:, 4:8, ds(c0, nt)])
                acc = None
                psc_last = None
                pend_sc = []
                th_big = thp.tile([P, HT, NT], BF16, tag="th", name="th")

                def main_group(ht):
                    pa1 = pa1_t[(ti * HT + ht) % 4]
                    for k in range(KP):
                        nc.tensor.matmul(
                            pa1[:, 0:nt],
                            w1e_t[ht][:, ds(2 * k, 2), :],
                            enc_sb[:, ds(2 * k, 2), 0:nt],
                            start=(k == 0), stop=(k == KP - 1),
                            perf_mode=DR,
                        )
                    return pa1

                # tile0 runs its first 4 PSUM groups before phase A so the PE
                # works on (early-arriving) enc while w1h is still in flight;
                # phase A's ACT/PE ops are emitted before any tanh, keeping
                # both in-order queues deadlock-free (tanh needs phase A's
                # bias).
                pa1_pend = {}
                if ti == 0:
                    for ht in range(4):
                        pa1_pend[ht] = main_group(ht)
                    phase_a()
                for ht in range(HT):
                    pa1 = pa1_pend.pop(ht, None)
                    if pa1 is None:
                        pa1 = main_group(ht)
                    # On the last tile, drain pending PE score matmuls two
                    # groups behind the tanh that feeds them.
                    if len(pend_sc) > 2:
                        pht_ = pend_sc.pop(0)
                        nc.tensor.matmul(psc_last[0:1, 0:nt],
                                         w2b_sb[:, ds(pht_, 1)],
                                         th_big[:, pht_, 0:nt],
                                         start=(pht_ == 0),
                                         stop=(pht_ == HT - 1))
                    for off, ln, b in segs:
                        nc.scalar.activation(th_big[:, ht, ds(off, ln)],
                                             pa1[:, ds(off, ln)], AF.Tanh,
                                             bias=bias_sb[:, ds(ht * BL + b, 1)],
                                             scale=1.0 / WS)
                    if last:
                        # last tile: w2 contraction on the PE (plain bf16,
                        # M=1) so the tail never waits on the DVE chain.
                        if ht == 0:
                            psc_last = pscp.tile([1, NT], F32, tag="psc",
                                                 name="psc")
                        pend_sc.append(ht)
                    elif ht == 0:
                        acc = accp.tile([P, NT], BF16, tag="acc", name="acc")
                        nc.vector.tensor_scalar_mul(acc[:, 0:nt],
                                                    th_big[:, 0, 0:nt],
                                                    w2T_sb[:, ds(0, 1)])
                    else:
                        nc.vector.scalar_tensor_tensor(
                            acc[:, 0:nt], th_big[:, ht, 0:nt],
                            w2T_sb[:, ds(ht, 1)], acc[:, 0:nt],
                            mybir.AluOpType.mult, mybir.AluOpType.add)
                    # Flush the previous tile's scores once this tile's PE
                    # pipeline is deep enough (never stalls the in-order PE).
                    if ht == 2 and prev is not None:
                        flush_scores(*prev)
                        prev = None
                if last:
                    for pht_ in pend_sc:
                        nc.tensor.matmul(psc_last[0:1, 0:nt],
                                         w2b_sb[:, ds(pht_, 1)],
                                         th_big[:, pht_, 0:nt],
                                         start=(pht_ == 0),
                                         stop=(pht_ == HT - 1))
                    if prev is not None:
                        flush_scores(*prev)
                        prev = None
                    flush_scores(ti, None, psc=psc_last)
                else:
                    prev = (ti, acc)

    nc.compile()
    _cached[CB] = (nc, TCOLS, tiles)
    return _cached[CB]


def kernel(hidden, encoder_outputs, mask, W1, b1, W2, b2):
    global LAST_RESULT

    mask = np.asarray(mask, dtype=bool)
    idx_all = [np.nonzero(~mask[gb])[0] for gb in range(B)]
    maxcnt = max(len(ix) for ix in idx_all)
    CB = max(576, -(-maxcnt // 64) * 64)
    nc, TCOLS, _ = _build(CB)

    enc = np.asarray(encoder_outputs, dtype=np.float32)
    # [S,B,Hin] -> [B,Hin,S] in fp8 so per-core DMAs are contiguous
    enc_t = np.ascontiguousarray(np.transpose(enc, (1, 2, 0)).astype(F8))
    hid_t = np.asarray(hidden, dtype=np.float32).T.astype(F8)  # [H=k, B]
    W1 = np.asarray(W1, dtype=np.float32)
    w1e8 = (WS * W1[:, :HIN].T).astype(F8)   # [K=HIN, H]
    w1h8 = (WS * W1[:, HIN:].T).astype(F8)   # [K=H, H]
    # w1e packed [ht, p, it, m] = w1e8[it*128+p, ht*128+m]
    w1e_pack = np.ascontiguousarray(
        w1e8.reshape(IT, P, HT, P).transpose(2, 1, 0, 3))
    # w1h packed [p, it, h] = w1h8[it*128+p, h]
    w1h_pack = np.ascontiguousarray(
        w1h8.reshape(IT, P, H).transpose(1, 0, 2))
    b1 = np.ascontiguousarray(np.asarray(b1, dtype=np.float32).reshape(H))
    # w2 packed [p, ht] = W2[ht*128+p], f32 per-partition scalars
    w2_pack = np.ascontiguousarray(
        np.asarray(W2, dtype=np.float32).reshape(HT, P).T)

    in_maps = []
    for c in range(N_CORES):
        sl = slice(c * BL, (c + 1) * BL)
        hid_pack = np.zeros((P, IT, BP), dtype=F8)
        hid_pack[:, :, 0:BL] = hid_t[:, sl].reshape(IT, P, BL).transpose(1, 0, 2)
        enc_pack = np.zeros((HIN, TCOLS), dtype=F8)
        mneg = np.full(TCOLS, -1e30, dtype=np.float32)
        for b in range(BL):
            ix = idx_all[c * BL + b]
            enc_pack[:, b * CB:b * CB + len(ix)] = enc_t[c * BL + b][:, ix]
            mneg[b * CB:b * CB + len(ix)] = 0.0
        in_maps.append({
            "enc": np.ascontiguousarray(enc_pack.reshape(IT, P, TCOLS)),
            "hiddent": hid_pack,
            "maskneg": mneg,
            "w1e": w1e_pack,
            "w1h": w1h_pack,
            "b1": b1,
            "w2": w2_pack,
        })

    res = run_bass_kernel_spmd(nc, in_maps, core_ids=list(range(N_CORES)))
    LAST_RESULT = res
    out = np.zeros((B, S), dtype=np.float32)
    for c in range(N_CORES):
        packed = res.results[c]["out"]
        for b in range(BL):
            gb = c * BL + b
            ix = idx_all[gb]
            out[gb, ix] = packed[b * CB:b * CB + len(ix)]
    return np.ascontiguousarray(out[:, None, :])


# revision 39
# speedup vs baseline: 1.1413x; 1.1413x over previous
"""Trainium2 Bass kernel for the attention-scoring MLP (nn_Attn):

    enc = encoder_outputs.transpose(1,0,2)          # [B,S,Hin]
    a1  = tanh(enc @ W1_enc.T + hidden @ W1_hid.T + b1)
    s   = a1 @ W2[0] (+ b2 -- dropped: softmax shift-invariant)
    s   = where(mask, -inf, s)
    out = softmax(s, axis=-1)[:, None, :]           # [B,1,S]

Sharding: data-parallel over batch B=32 across 8 NeuronCores (4 rows
each), weights replicated, no collectives.

Mask packing: masked positions get score -inf and contribute nothing to
the softmax, so the kernel only computes the ~50% unmasked columns.
kernel() gathers each row's unmasked enc columns into a packed layout of
CB columns per row (CB = 576 covers the binomial spread; zero columns +
-1e30 mask data pad the remainder, so the instruction stream is
SPMD-uniform across cores -- only DMA'd data differs). The host scatters
the packed attention weights back to full [B,1,S] (pure layout, no
arithmetic).

Per core the main matmul is computed transposed -- a1T[h, s] =
W1_encT.T @ encT -- so the (b1 + hidden@W1_hid.T) term rides the
ScalarEngine's per-partition bias port of the tanh activation. Matmuls
run in fp8 (e4m3) with MatmulPerfMode.DoubleRow: each matmul contracts a
PAIR of 128-row k-tiles (lhsT/rhs laid out [128, 2, n]), the PE
streaming 2 fp8 rows/cycle. Weights are pre-scaled by 32 host-side so
their +-1/sqrt(2048) range sits in fp8's normal range; the 1/32 is
folded into the tanh activation scale. Accumulation is fp32 in PSUM.
Each tile's enc arrives in ONE DMA instruction (descriptor generation at
~600ns/instruction was throttling the pipeline when split per k-tile).

The w2 score contraction runs as an in-place DVE accumulate chain over
ht (acc = w2[:,ht]*th_ht + acc, per-partition scalars) plus a single
bf16 ones-matmul to reduce partitions; the final tile instead issues M=1
score matmuls on the PE so the tail never waits on the DVE chain.
"""

import numpy as np
import ml_dtypes

import concourse.bass as bass
import concourse.tile as tile
from concourse import bacc, mybir
from concourse.bass import ds, ts
from concourse.bass_utils import run_bass_kernel_spmd
from concourse.masks import make_identity

N_CORES = 8
B, S, HIN, H = 32, 1024, 1024, 1024
BL = B // N_CORES          # local batch rows per core
P = 128                    # partitions
IT = HIN // P              # contraction k-tiles
KP = IT // 2               # k-tile pairs (DoubleRow)
HT = H // P                # output-feature tiles
NT = 512                   # moving-dim tile (s columns per matmul)
BP = 16                    # padded batch rows (dual-fp8 ldweights step%16)
WS = 32.0                  # host-side weight scale (undone in act scale)
F32 = mybir.dt.float32
BF16 = mybir.dt.bfloat16
FP8 = mybir.dt.float8e4
AF = mybir.ActivationFunctionType
DR = mybir.MatmulPerfMode.DoubleRow
F8 = ml_dtypes.float8_e4m3

_cached = {}
LAST_RESULT = None  # BassKernelResults of the most recent run (for test harness)


def _layout(CB):
    """Static tile/segment layout for packed width CB (multiple of 64)."""
    TCOLS = BL * CB
    tiles = []  # (col0, nt, [(off, ln, b), ...])
    col = 0
    while col < TCOLS:
        nt = min(NT, TCOLS - col)
        segs = []
        o = col
        while o < col + nt:
            b = o // CB
            end = min((b + 1) * CB, col + nt)
            segs.append((o - col, end - o, b))
            o = end
        tiles.append((col, nt, segs))
        col += nt
    slot_of = {}
    b_slots = {b: [] for b in range(BL)}
    s = 0
    for ti, (_, _, segs) in enumerate(tiles):
        for si, (_, _, b) in enumerate(segs):
            slot_of[(ti, si)] = s
            b_slots[b].append(s)
            s += 1
    return TCOLS, tiles, slot_of, b_slots, s


def _build(CB):
    if CB in _cached:
        return _cached[CB]
    TCOLS, tiles, slot_of, b_slots, NSEG = _layout(CB)
    NTI = len(tiles)

    nc = bacc.Bacc("TRN2", target_bir_lowering=False, debug=False,
                   num_devices=N_CORES)

    # packed encT: [it, p, col] fp8 (k = it*128+p; col = b*CB + packed s)
    enc_ext = nc.dram_tensor("enc", [IT, P, TCOLS], FP8, kind="ExternalInput").ap()
    # hiddenT packed+padded [p, it, bp] fp8 (k = it*128+p; bp 0..3 real)
    hidt_ext = nc.dram_tensor("hiddent", [P, IT, BP], FP8, kind="ExternalInput").ap()
    mneg_ext = nc.dram_tensor("maskneg", [TCOLS], F32, kind="ExternalInput").ap()
    # W1_enc.T packed per ht: [ht, p, it, m] = 32*W1[ht*128+m, it*128+p]
    w1e_ext = nc.dram_tensor("w1e", [HT, P, IT, P], FP8, kind="ExternalInput").ap()
    # W1_hid.T packed: [p, it, h]
    w1h_ext = nc.dram_tensor("w1h", [P, IT, H], FP8, kind="ExternalInput").ap()
    b1_ext = nc.dram_tensor("b1", [H], F32, kind="ExternalInput").ap()
    # W2 packed [p, ht] f32 (h = ht*128+p)
    w2_ext = nc.dram_tensor("w2", [P, HT], F32, kind="ExternalInput").ap()
    out_ext = nc.dram_tensor("out", [TCOLS], F32, kind="ExternalOutput").ap()

    encR = enc_ext.rearrange("it p c -> p it c")

    with tile.TileContext(nc) as tc:
        with (
            tc.tile_pool(name="consts", bufs=1) as consts,
            tc.tile_pool(name="encp", bufs=3) as encp,
            tc.tile_pool(name="thp", bufs=2) as thp,
            tc.tile_pool(name="accp", bufs=2) as accp,
            tc.tile_pool(name="pap", bufs=1, space="PSUM") as pap,
            tc.tile_pool(name="pscp", bufs=2, space="PSUM") as pscp,
            tc.tile_pool(name="psA", bufs=1, space="PSUM") as psA,
            tc.tile_pool(name="psT", bufs=1, space="PSUM") as psTp,
        ):
            # ---- resident weights/constants. A single DMA instruction runs
            # on ONE queue at ~68 GB/s, so the startup-critical transfers are
            # split across several rings/queues to run on parallel DMA
            # engines, ordered by deadline (w1e0+enc0 gate tile0's first
            # group, w1h gates phase A after tile0's 4th group).
            w1e_t = []
            w = consts.tile([P, IT, P], FP8, tag="w1e0", name="w1e0")
            nc.sync.dma_start(w[:], w1e_ext[0])
            w1e_t.append(w)
            enc0_sb = encp.tile([P, IT, NT], FP8, tag="enc", name="enc")
            nt0 = tiles[0][1]
            nc.scalar.dma_start(enc0_sb[:, 0:3, 0:nt0], encR[:, 0:3, ds(0, nt0)])
            nc.gpsimd.dma_start(enc0_sb[:, 3:6, 0:nt0], encR[:, 3:6, ds(0, nt0)])
            nc.sync.dma_start(enc0_sb[:, 6:8, 0:nt0], encR[:, 6:8, ds(0, nt0)])
            hT_sb = consts.tile([P, IT, BP], FP8)
            nc.sync.dma_start(hT_sb[:], hidt_ext[:])
            # w1h in halves on two rings (gates phase A)
            w1h_sb = consts.tile([P, IT, H], FP8)
            nc.scalar.dma_start(w1h_sb[:, 0:4, :], w1h_ext[:, 0:4, :])
            nc.gpsimd.dma_start(w1h_sb[:, 4:8, :], w1h_ext[:, 4:8, :])
            for ht in range(1, HT):
                w = consts.tile([P, IT, P], FP8, tag=f"w1e{ht}", name=f"w1e{ht}")
                nc.sync.dma_start(w[:], w1e_ext[ht])
                w1e_t.append(w)
            b1T_sb = consts.tile([P, HT], F32)
            nc.sync.dma_start(b1T_sb[:], b1_ext.rearrange("(ht p) -> p ht", p=P))
            w2T_sb = consts.tile([P, HT], F32)
            nc.sync.dma_start(w2T_sb[:], w2_ext[:])
            mneg_sb = consts.tile([1, TCOLS], F32)
            nc.sync.dma_start(mneg_sb[:], mneg_ext[:])

            # ---- PE warmup: junk matmuls with no DMA deps so the HAM
            # clock-gate / p-state ramp toward full speed during the preamble.
            warm_sb = consts.tile([P, NT], BF16)
            nc.gpsimd.memset(warm_sb[:], 0.0)
            # 4 persistent PSUM accumulators, rotated manually (tile-release
            # bookkeeping at kernel end costs ~30ns/allocation across the
            # engines' semaphore queues, so allocate once).
            pa1_t = [pap.tile([P, NT], F32, tag=f"pa1_{i}", name=f"pa1_{i}")
                     for i in range(4)]
            for _ in range(10):
                nc.tensor.matmul(pa1_t[0][:], warm_sb[:, 0:P], warm_sb[:],
                                 start=True, stop=True)

            ident_sb = consts.tile([BL, BL], F32)
            make_identity(nc, ident_sb[:])
            ones_sb = consts.tile([P, 1], BF16)
            nc.gpsimd.memset(ones_sb[:], 1.0)
            w2b_sb = consts.tile([P, HT], BF16)
            nc.vector.tensor_copy(w2b_sb[:], w2T_sb[:])

            bias_sb = consts.tile([P, HT * BL], F32)   # [p, ht*BL+b]
            hterm_sb = consts.tile([BL, H], F32)
            scores_sb = consts.tile([1, TCOLS], F32)
            c40 = consts.tile([1, 1], F32)
            nc.gpsimd.memset(c40[:], -40.0)
            exps = consts.tile([1, TCOLS], F32)
            ssum = consts.tile([1, NSEG], F32)
            rcp = consts.tile([1, BL], F32)
            attn = consts.tile([1, TCOLS], F32)

            def phase_a():
                # h_term[b,h] = hidden @ W1_hid.T (x32 in fp8, DoubleRow with
                # the batch dim padded to 16 so the dual-fp8 ldweights pair
                # step is 16); bias = h_termT/32 + b1T.
                ptT = psTp.tile([P, HT * BL], F32)
                for g in range(2):
                    pht = psA.tile([BP, NT], F32, tag="pht", name="pht")
                    for k in range(KP):
                        nc.tensor.matmul(pht[:],
                                         hT_sb[:, ds(2 * k, 2), :],
                                         w1h_sb[:, ds(2 * k, 2), ds(g * NT, NT)],
                                         start=(k == 0), stop=(k == KP - 1),
                                         perf_mode=DR)
                    nc.scalar.mul(hterm_sb[:, ds(g * NT, NT)], pht[0:BL, :],
                                  1.0 / WS)
                for ht in range(HT):
                    nc.tensor.transpose(ptT[:, ts(ht, BL)],
                                        hterm_sb[:, ts(ht, P)], ident_sb[:])
                    nc.vector.tensor_scalar_add(bias_sb[:, ts(ht, BL)],
                                                ptT[:, ts(ht, BL)],
                                                b1T_sb[:, ds(ht, 1)])

            # ---- phase B ----
            def flush_scores(ti, acc_f, psc=None):
                c0, nt, segs = tiles[ti]
                if psc is None:
                    psc = pscp.tile([1, NT], F32, tag="psc", name="psc")
                    nc.tensor.matmul(psc[0:1, 0:nt], ones_sb[:],
                                     acc_f[:, 0:nt], start=True, stop=True)
                # scores += mask * -1e30 (also kills the packing pad columns)
                nc.vector.tensor_add(scores_sb[0:1, ds(c0, nt)],
                                     psc[0:1, 0:nt],
                                     mneg_sb[0:1, ds(c0, nt)])
                # |scores| <= ||W2||_1 <= 32: exp(s - 40) never overflows and
                # softmax is shift-invariant -- no max-reduce needed.
                for si, (off, ln, b) in enumerate(segs):
                    slot = slot_of[(ti, si)]
                    nc.scalar.activation(exps[0:1, ds(c0 + off, ln)],
                                         scores_sb[0:1, ds(c0 + off, ln)],
                                         AF.Exp, bias=c40[0:1, 0:1], scale=1.0)
                    nc.vector.reduce_sum(ssum[0:1, ds(slot, 1)],
                                         exps[0:1, ds(c0 + off, ln)],
                                         axis=mybir.AxisListType.X)
                    if slot == b_slots[b][-1]:
                        s0 = b_slots[b][0]
                        nsl = len(b_slots[b])
                        nc.vector.reduce_sum(rcp[0:1, ds(b, 1)],
                                             ssum[0:1, ds(s0, nsl)],
                                             axis=mybir.AxisListType.X)
                        nc.vector.reciprocal(rcp[0:1, ds(b, 1)],
                                             rcp[0:1, ds(b, 1)])
                        nc.vector.tensor_scalar_mul(attn[0:1, ds(b * CB, CB)],
                                                    exps[0:1, ds(b * CB, CB)],
                                                    rcp[0:1, ds(b, 1)])
                        nc.sync.dma_start(out_ext[ds(b * CB, CB)],
                                          attn[0:1, ds(b * CB, CB)])

            prev = None  # (tile idx, final acc tile) awaiting score flush
            for ti, (c0, nt, segs) in enumerate(tiles):
                last = ti == NTI - 1
                if ti == 0:
                    enc_sb = enc0_sb
                else:
                    # two half-tile DMAs on different rings -> two queues in
                    # parallel (~3.8us/tile), triggered ~2 tiles ahead of use.
                    enc_sb = encp.tile([P, IT, NT], FP8, tag="enc", name="enc")
                    ea, eb = ((nc.scalar, nc.gpsimd) if ti == 1
                              else (nc.sync, nc.gpsimd))
                    ea.dma_start(enc_sb[:, 0:4, 0:nt], encR[:, 0:4, ds(c0, nt)])
                    eb.dma_start(enc_sb[:, 4:8, 0:nt], encR[:, 4:8, ds(c0, nt)])
                acc = None
                psc_last = None
                pend_sc = []
                th_big = thp.tile([P, HT, NT], BF16, tag="th", name="th")

                def main_group(ht):
                    pa1 = pa1_t[(ti * HT + ht) % 4]
                    for k in range(KP):
                        nc.tensor.matmul(
                            pa1[:, 0:nt],
                            w1e_t[ht][:, ds(2 * k, 2), :],
                            enc_sb[:, ds(2 * k, 2), 0:nt],
                            start=(k == 0), stop=(k == KP - 1),
                            perf_mode=DR,
                        )
                    return pa1

                # tile0 runs its first 4 PSUM groups before phase A so the PE
                # works on (early-arriving) enc while w1h is still in flight;
                # phase A's ACT/PE ops are emitted before any tanh, keeping
                # both in-order queues deadlock-free (tanh needs phase A's
                # bias).
                pa1_pend = {}
                if ti == 0:
                    for ht in range(4):
                        pa1_pend[ht] = main_group(ht)
                    phase_a()
                for ht in range(HT):
                    pa1 = pa1_pend.pop(ht, None)
                    if pa1 is None:
                        pa1 = main_group(ht)
                    # On the last tile, drain pending PE score matmuls two
                    # groups behind the tanh that feeds them.
                    if len(pend_sc) > 2:
                        pht_ = pend_sc.pop(0)
                        nc.tensor.matmul(psc_last[0:1, 0:nt],
                                         w2b_sb[:, ds(pht_, 1)],
                                         th_big[:, pht_, 0:nt],
                                         start=(pht_ == 0),
                                         stop=(pht_ == HT - 1))
                    for off, ln, b in segs:
                        nc.scalar.activation(th_big[:, ht, ds(off, ln)],
                                             pa1[:, ds(off, ln)], AF.Tanh,
                                             bias=bias_sb[:, ds(ht * BL + b, 1)],
                                             scale=1.0 / WS)
                    if last:
                        # last tile: w2 contraction on the PE (plain bf16,
                        # M=1) so the tail never waits on the DVE chain.
                        if ht == 0:
                            psc_last = pscp.tile([1, NT], F32, tag="psc",
                                                 name="psc")
                        pend_sc.append(ht)
                    elif ht == 0:
                        acc = accp.tile([P, NT], BF16, tag="acc", name="acc")
                        nc.vector.tensor_scalar_mul(acc[:, 0:nt],
                                                    th_big[:, 0, 0:nt],
                                                    w2T_sb[:, ds(0, 1)])
                    else:
                        nc.vector.scalar_tensor_tensor(
                            acc[:, 0:nt], th_big[:, ht, 0:nt],
                            w2T_sb[:, ds(ht, 1)], acc[:, 0:nt],
                            mybir.AluOpType.mult, mybir.AluOpType.add)
                    # Flush the previous tile's scores once this tile's PE
                    # pipeline is deep enough (never stalls the in-order PE).
                    if ht == 2 and prev is not None:
                        flush_scores(*prev)
                        prev = None
                if last:
                    for pht_ in pend_sc:
                        nc.tensor.matmul(psc_last[0:1, 0:nt],
                                         w2b_sb[:, ds(pht_, 1)],
                                         th_big[:, pht_, 0:nt],
                                         start=(pht_ == 0),
                                         stop=(pht_ == HT - 1))
                    if prev is not None:
                        flush_scores(*prev)
                        prev = None
                    flush_scores(ti, None, psc=psc_last)
                else:
                    prev = (ti, acc)

    nc.compile()
    _cached[CB] = (nc, TCOLS, tiles)
    return _cached[CB]


def kernel(hidden, encoder_outputs, mask, W1, b1, W2, b2):
    global LAST_RESULT

    mask = np.asarray(mask, dtype=bool)
    idx_all = [np.nonzero(~mask[gb])[0] for gb in range(B)]
    maxcnt = max(len(ix) for ix in idx_all)
    CB = max(576, -(-maxcnt // 64) * 64)
    nc, TCOLS, _ = _build(CB)

    enc = np.asarray(encoder_outputs, dtype=np.float32)
    # [S,B,Hin] -> [B,Hin,S] in fp8 so per-core DMAs are contiguous
    enc_t = np.ascontiguousarray(np.transpose(enc, (1, 2, 0)).astype(F8))
    hid_t = np.asarray(hidden, dtype=np.float32).T.astype(F8)  # [H=k, B]
    W1 = np.asarray(W1, dtype=np.float32)
    w1e8 = (WS * W1[:, :HIN].T).astype(F8)   # [K=HIN, H]
    w1h8 = (WS * W1[:, HIN:].T).astype(F8)   # [K=H, H]
    # w1e packed [ht, p, it, m] = w1e8[it*128+p, ht*128+m]
    w1e_pack = np.ascontiguousarray(
        w1e8.reshape(IT, P, HT, P).transpose(2, 1, 0, 3))
    # w1h packed [p, it, h] = w1h8[it*128+p, h]
    w1h_pack = np.ascontiguousarray(
        w1h8.reshape(IT, P, H).transpose(1, 0, 2))
    b1 = np.ascontiguousarray(np.asarray(b1, dtype=np.float32).reshape(H))
    # w2 packed [p, ht] = W2[ht*128+p], f32 per-partition scalars
    w2_pack = np.ascontiguousarray(
        np.asarray(W2, dtype=np.float32).reshape(HT, P).T)

    in_maps = []
    for c in range(N_CORES):
        sl = slice(c * BL, (c + 1) * BL)
        hid_pack = np.zeros((P, IT, BP), dtype=F8)
        hid_pack[:, :, 0:BL] = hid_t[:, sl].reshape(IT, P, BL).transpose(1, 0, 2)
        enc_pack = np.zeros((HIN, TCOLS), dtype=F8)
        mneg = np.full(TCOLS, -1e30, dtype=np.float32)
        for b in range(BL):
            ix = idx_all[c * BL + b]
            enc_pack[:, b * CB:b * CB + len(ix)] = enc_t[c * BL + b][:, ix]
            mneg[b * CB:b * CB + len(ix)] = 0.0
        in_maps.append({
            "enc": np.ascontiguousarray(enc_pack.reshape(IT, P, TCOLS)),
            "hiddent": hid_pack,
            "maskneg": mneg,
            "w1e": w1e_pack,
            "w1h": w1h_pack,
            "b1": b1,
            "w2": w2_pack,
        })

    res = run_bass_kernel_spmd(nc, in_maps, core_ids=list(range(N_CORES)))
    LAST_RESULT = res
    out = np.zeros((B, S), dtype=np.float32)
    for c in range(N_CORES):
        packed = res.results[c]["out"]
        for b in range(BL):
            gb = c * BL + b
            ix = idx_all[gb]
            out[gb, ix] = packed[b * CB:b * CB + len(ix)]
    return np.ascontiguousarray(out[:, None, :])


# revision 41
# speedup vs baseline: 1.2384x; 1.0851x over previous
"""Trainium2 Bass kernel for the attention-scoring MLP (nn_Attn):

    enc = encoder_outputs.transpose(1,0,2)          # [B,S,Hin]
    a1  = tanh(enc @ W1_enc.T + hidden @ W1_hid.T + b1)
    s   = a1 @ W2[0] (+ b2 -- dropped: softmax shift-invariant)
    s   = where(mask, -inf, s)
    out = softmax(s, axis=-1)[:, None, :]           # [B,1,S]

Sharding: data-parallel over batch B=32 across 8 NeuronCores (4 rows
each), weights replicated, no collectives.

Mask packing: masked positions get score -inf and contribute nothing to
the softmax, so the kernel only computes the ~50% unmasked columns.
kernel() gathers each row's unmasked enc columns into a packed layout of
CB columns per row (CB = 576 covers the binomial spread; zero columns +
-1e30 mask data pad the remainder, so the instruction stream is
SPMD-uniform across cores -- only DMA'd data differs). The host scatters
the packed attention weights back to full [B,1,S] (pure layout, no
arithmetic).

Per core the main matmul is computed transposed -- a1T[h, s] =
W1_encT.T @ encT -- so the (b1 + hidden@W1_hid.T) term rides the
ScalarEngine's per-partition bias port of the tanh activation. Matmuls
run in fp8 (e4m3) with MatmulPerfMode.DoubleRow: each matmul contracts a
PAIR of 128-row k-tiles (lhsT/rhs laid out [128, 2, n]), the PE
streaming 2 fp8 rows/cycle. Weights are pre-scaled by 32 host-side so
their +-1/sqrt(2048) range sits in fp8's normal range; the 1/32 is
folded into the tanh activation scale. Accumulation is fp32 in PSUM.
Each tile's enc arrives in ONE DMA instruction (descriptor generation at
~600ns/instruction was throttling the pipeline when split per k-tile).

The w2 score contraction runs as an in-place DVE accumulate chain over
ht (acc = w2[:,ht]*th_ht + acc, per-partition scalars) plus a single
bf16 ones-matmul to reduce partitions; the final tile instead issues M=1
score matmuls on the PE so the tail never waits on the DVE chain.
"""

import numpy as np
import ml_dtypes

import concourse.bass as bass
import concourse.tile as tile
from concourse import bacc, mybir
from concourse.bass import ds, ts
from concourse.bass_utils import run_bass_kernel_spmd
from concourse.masks import make_identity

N_CORES = 8
B, S, HIN, H = 32, 1024, 1024, 1024
BL = B // N_CORES          # local batch rows per core
P = 128                    # partitions
IT = HIN // P              # contraction k-tiles
KP = IT // 2               # k-tile pairs (DoubleRow)
HT = H // P                # output-feature tiles
NT = 512                   # moving-dim tile (s columns per matmul)
BP = 16                    # padded batch rows (dual-fp8 ldweights step%16)
WS = 32.0                  # host-side weight scale (undone in act scale)
F32 = mybir.dt.float32
BF16 = mybir.dt.bfloat16
FP8 = mybir.dt.float8e4
AF = mybir.ActivationFunctionType
DR = mybir.MatmulPerfMode.DoubleRow
F8 = ml_dtypes.float8_e4m3

_cached = {}
LAST_RESULT = None  # BassKernelResults of the most recent run (for test harness)


def _layout(CB):
    """Static tile/segment layout for packed width CB (multiple of 64)."""
    TCOLS = BL * CB
    tiles = []  # (col0, nt, [(off, ln, b), ...])
    col = 0
    while col < TCOLS:
        nt = min(NT, TCOLS - col)
        segs = []
        o = col
        while o < col + nt:
            b = o // CB
            end = min((b + 1) * CB, col + nt)
            segs.append((o - col, end - o, b))
            o = end
        tiles.append((col, nt, segs))
        col += nt
    slot_of = {}
    b_slots = {b: [] for b in range(BL)}
    s = 0
    for ti, (_, _, segs) in enumerate(tiles):
        for si, (_, _, b) in enumerate(segs):
            slot_of[(ti, si)] = s
            b_slots[b].append(s)
            s += 1
    return TCOLS, tiles, slot_of, b_slots, s


def _build(CB):
    if CB in _cached:
        return _cached[CB]
    TCOLS, tiles, slot_of, b_slots, NSEG = _layout(CB)
    NTI = len(tiles)

    nc = bacc.Bacc("TRN2", target_bir_lowering=False, debug=False,
                   num_devices=N_CORES)

    # packed encT: [it, p, col] fp8 (k = it*128+p; col = b*CB + packed s)
    enc_ext = nc.dram_tensor("enc", [IT, P, TCOLS], FP8, kind="ExternalInput").ap()
    # hiddenT packed+padded [p, it, bp] fp8 (k = it*128+p; bp 0..3 real)
    hidt_ext = nc.dram_tensor("hiddent", [P, IT, BP], FP8, kind="ExternalInput").ap()
    mneg_ext = nc.dram_tensor("maskneg", [TCOLS], F32, kind="ExternalInput").ap()
    # W1_enc.T packed per ht: [ht, p, it, m] = 32*W1[ht*128+m, it*128+p]
    w1e_ext = nc.dram_tensor("w1e", [HT, P, IT, P], FP8, kind="ExternalInput").ap()
    # W1_hid.T packed: [p, it, h]
    w1h_ext = nc.dram_tensor("w1h", [P, IT, H], FP8, kind="ExternalInput").ap()
    b1_ext = nc.dram_tensor("b1", [H], F32, kind="ExternalInput").ap()
    # W2 packed [p, ht] f32 (h = ht*128+p)
    w2_ext = nc.dram_tensor("w2", [P, HT], F32, kind="ExternalInput").ap()
    out_ext = nc.dram_tensor("out", [TCOLS], F32, kind="ExternalOutput").ap()

    encR = enc_ext.rearrange("it p c -> p it c")

    with tile.TileContext(nc) as tc:
        with (
            tc.tile_pool(name="consts", bufs=1) as consts,
            tc.tile_pool(name="encp", bufs=3) as encp,
            tc.tile_pool(name="thp", bufs=2) as thp,
            tc.tile_pool(name="accp", bufs=2) as accp,
            tc.tile_pool(name="pap", bufs=1, space="PSUM") as pap,
            tc.tile_pool(name="pscp", bufs=2, space="PSUM") as pscp,
            tc.tile_pool(name="psA", bufs=1, space="PSUM") as psA,
            tc.tile_pool(name="psT", bufs=1, space="PSUM") as psTp,
        ):
            # ---- resident weights/constants. A single DMA instruction runs
            # on ONE queue at ~68 GB/s, so the startup-critical transfers are
            # split across several rings/queues to run on parallel DMA
            # engines, ordered by deadline (w1e0+enc0 gate tile0's first
            # group, w1h gates phase A after tile0's 4th group).
            w1e_t = []
            w = consts.tile([P, IT, P], FP8, tag="w1e0", name="w1e0")
            nc.sync.dma_start(w[:], w1e_ext[0])
            w1e_t.append(w)
            hT_sb = consts.tile([P, IT, BP], FP8)
            nc.sync.dma_start(hT_sb[:], hidt_ext[:])
            enc0_sb = encp.tile([P, IT, NT], FP8, tag="enc", name="enc")
            nt0 = tiles[0][1]
            nc.scalar.dma_start(enc0_sb[:, :, 0:nt0], encR[:, :, ds(0, nt0)])
            w1h_sb = consts.tile([P, IT, H], FP8)
            nc.scalar.dma_start(w1h_sb[:], w1h_ext[:])
            for ht in range(1, HT):
                w = consts.tile([P, IT, P], FP8, tag=f"w1e{ht}", name=f"w1e{ht}")
                nc.sync.dma_start(w[:], w1e_ext[ht])
                w1e_t.append(w)
            b1T_sb = consts.tile([P, HT], F32)
            nc.sync.dma_start(b1T_sb[:], b1_ext.rearrange("(ht p) -> p ht", p=P))
            w2T_sb = consts.tile([P, HT], F32)
            nc.sync.dma_start(w2T_sb[:], w2_ext[:])
            mneg_sb = consts.tile([1, TCOLS], F32)
            nc.sync.dma_start(mneg_sb[:], mneg_ext[:])

            # ---- PE warmup: junk matmuls with no DMA deps so the HAM
            # clock-gate / p-state ramp toward full speed during the preamble.
            warm_sb = consts.tile([P, NT], BF16)
            nc.gpsimd.memset(warm_sb[:], 0.0)
            # 4 persistent PSUM accumulators, rotated manually (tile-release
            # bookkeeping at kernel end costs ~30ns/allocation across the
            # engines' semaphore queues, so allocate once).
            pa1_t = [pap.tile([P, NT], F32, tag=f"pa1_{i}", name=f"pa1_{i}")
                     for i in range(4)]
            for _ in range(10):
                nc.tensor.matmul(pa1_t[0][:], warm_sb[:, 0:P], warm_sb[:],
                                 start=True, stop=True)

            ident_sb = consts.tile([BL, BL], F32)
            make_identity(nc, ident_sb[:])
            ones_sb = consts.tile([P, 1], BF16)
            nc.gpsimd.memset(ones_sb[:], 1.0)
            w2b_sb = consts.tile([P, HT], BF16)
            nc.vector.tensor_copy(w2b_sb[:], w2T_sb[:])

            bias_sb = consts.tile([P, HT * BL], F32)   # [p, ht*BL+b]
            hterm_sb = consts.tile([BL, H], F32)
            scores_sb = consts.tile([1, TCOLS], F32)
            c40 = consts.tile([1, 1], F32)
            nc.gpsimd.memset(c40[:], -40.0)
            exps = consts.tile([1, TCOLS], F32)
            ssum = consts.tile([1, NSEG], F32)
            rcp = consts.tile([1, BL], F32)
            attn = consts.tile([1, TCOLS], F32)

            def phase_a():
                # h_term[b,h] = hidden @ W1_hid.T (x32 in fp8, DoubleRow with
                # the batch dim padded to 16 so the dual-fp8 ldweights pair
                # step is 16); bias = h_termT/32 + b1T.
                ptT = psTp.tile([P, HT * BL], F32)
                for g in range(2):
                    pht = psA.tile([BP, NT], F32, tag="pht", name="pht")
                    for k in range(KP):
                        nc.tensor.matmul(pht[:],
                                         hT_sb[:, ds(2 * k, 2), :],
                                         w1h_sb[:, ds(2 * k, 2), ds(g * NT, NT)],
                                         start=(k == 0), stop=(k == KP - 1),
                                         perf_mode=DR)
                    nc.scalar.mul(hterm_sb[:, ds(g * NT, NT)], pht[0:BL, :],
                                  1.0 / WS)
                for ht in range(HT):
                    nc.tensor.transpose(ptT[:, ts(ht, BL)],
                                        hterm_sb[:, ts(ht, P)], ident_sb[:])
                    nc.vector.tensor_scalar_add(bias_sb[:, ts(ht, BL)],
                                                ptT[:, ts(ht, BL)],
                                                b1T_sb[:, ds(ht, 1)])

            # ---- phase B ----
            def flush_scores(ti, acc_f, psc=None):
                c0, nt, segs = tiles[ti]
                if psc is None:
                    psc = pscp.tile([1, NT], F32, tag="psc", name="psc")
                    nc.tensor.matmul(psc[0:1, 0:nt], ones_sb[:],
                                     acc_f[:, 0:nt], start=True, stop=True)
                # scores += mask * -1e30 (also kills the packing pad columns)
                nc.vector.tensor_add(scores_sb[0:1, ds(c0, nt)],
                                     psc[0:1, 0:nt],
                                     mneg_sb[0:1, ds(c0, nt)])
                # |scores| <= ||W2||_1 <= 32: exp(s - 40) never overflows and
                # softmax is shift-invariant -- no max-reduce needed.
                for si, (off, ln, b) in enumerate(segs):
                    slot = slot_of[(ti, si)]
                    nc.scalar.activation(exps[0:1, ds(c0 + off, ln)],
                                         scores_sb[0:1, ds(c0 + off, ln)],
                                         AF.Exp, bias=c40[0:1, 0:1], scale=1.0)
                    nc.vector.reduce_sum(ssum[0:1, ds(slot, 1)],
                                         exps[0:1, ds(c0 + off, ln)],
                                         axis=mybir.AxisListType.X)
                    if slot == b_slots[b][-1]:
                        s0 = b_slots[b][0]
                        nsl = len(b_slots[b])
                        nc.vector.reduce_sum(rcp[0:1, ds(b, 1)],
                                             ssum[0:1, ds(s0, nsl)],
                                             axis=mybir.AxisListType.X)
                        nc.vector.reciprocal(rcp[0:1, ds(b, 1)],
                                             rcp[0:1, ds(b, 1)])
                        nc.vector.tensor_scalar_mul(attn[0:1, ds(b * CB, CB)],
                                                    exps[0:1, ds(b * CB, CB)],
                                                    rcp[0:1, ds(b, 1)])
                        nc.sync.dma_start(out_ext[ds(b * CB, CB)],
                                          attn[0:1, ds(b * CB, CB)])

            prev = None  # (tile idx, final acc tile) awaiting score flush
            for ti, (c0, nt, segs) in enumerate(tiles):
                last = ti == NTI - 1
                if ti == 0:
                    enc_sb = enc0_sb
                else:
                    enc_sb = encp.tile([P, IT, NT], FP8, tag="enc", name="enc")
                    eng = nc.scalar if ti == 1 else nc.sync
                    eng.dma_start(enc_sb[:, :, 0:nt], encR[:, :, ds(c0, nt)])
                acc = None
                psc_last = None
                pend_sc = []
                th_big = thp.tile([P, HT, NT], BF16, tag="th", name="th")

                def main_group(ht):
                    pa1 = pa1_t[(ti * HT + ht) % 4]
                    for k in range(KP):
                        nc.tensor.matmul(
                            pa1[:, 0:nt],
                            w1e_t[ht][:, ds(2 * k, 2), :],
                            enc_sb[:, ds(2 * k, 2), 0:nt],
                            start=(k == 0), stop=(k == KP - 1),
                            perf_mode=DR,
                        )
                    return pa1

                # tile0 runs its first 4 PSUM groups before phase A so the PE
                # works on (early-arriving) enc while w1h is still in flight;
                # phase A's ACT/PE ops are emitted before any tanh, keeping
                # both in-order queues deadlock-free (tanh needs phase A's
                # bias).
                pa1_pend = {}
                if ti == 0:
                    for ht in range(4):
                        pa1_pend[ht] = main_group(ht)
                    phase_a()
                for ht in range(HT):
                    pa1 = pa1_pend.pop(ht, None)
                    if pa1 is None:
                        pa1 = main_group(ht)
                    # On the last tile, drain pending PE score matmuls two
                    # groups behind the tanh that feeds them.
                    if len(pend_sc) > 2:
                        pht_ = pend_sc.pop(0)
                        nc.tensor.matmul(psc_last[0:1, 0:nt],
                                         w2b_sb[:, ds(pht_, 1)],
                                         th_big[:, pht_, 0:nt],
                                         start=(pht_ == 0),
                                         stop=(pht_ == HT - 1))
                    for off, ln, b in segs:
                        nc.scalar.activation(th_big[:, ht, ds(off, ln)],
                                             pa1[:, ds(off, ln)], AF.Tanh,
                                             bias=bias_sb[:, ds(ht * BL + b, 1)],
                                             scale=1.0 / WS)
                    if last:
                        # last tile: w2 contraction on the PE (plain bf16,
                        # M=1) so the tail never waits on the DVE chain.
                        if ht == 0:
                            psc_last = pscp.tile([1, NT], F32, tag="psc",
                                                 name="psc")
                        pend_sc.append(ht)
                    elif ht == 0:
                        acc = accp.tile([P, NT], BF16, tag="acc", name="acc")
                        nc.vector.tensor_scalar_mul(acc[:, 0:nt],
                                                    th_big[:, 0, 0:nt],
                                                    w2T_sb[:, ds(0, 1)])
                    else:
                        nc.vector.scalar_tensor_tensor(
                            acc[:, 0:nt], th_big[:, ht, 0:nt],
                            w2T_sb[:, ds(ht, 1)], acc[:, 0:nt],
                            mybir.AluOpType.mult, mybir.AluOpType.add)
                    # Flush the previous tile's scores once this tile's PE
                    # pipeline is deep enough (never stalls the in-order PE).
                    if ht == 2 and prev is not None:
                        flush_scores(*prev)
                        prev = None
                if last:
                    for pht_ in pend_sc:
                        nc.tensor.matmul(psc_last[0:1, 0:nt],
                                         w2b_sb[:, ds(pht_, 1)],
                                         th_big[:, pht_, 0:nt],
                                         start=(pht_ == 0),
                                         stop=(pht_ == HT - 1))
                    if prev is not None:
                        flush_scores(*prev)
                        prev = None
                    flush_scores(ti, None, psc=psc_last)
                else:
                    prev = (ti, acc)

    nc.compile()
    _cached[CB] = (nc, TCOLS, tiles)
    return _cached[CB]


def kernel(hidden, encoder_outputs, mask, W1, b1, W2, b2):
    global LAST_RESULT

    mask = np.asarray(mask, dtype=bool)
    idx_all = [np.nonzero(~mask[gb])[0] for gb in range(B)]
    maxcnt = max(len(ix) for ix in idx_all)
    CB = max(576, -(-maxcnt // 64) * 64)
    nc, TCOLS, _ = _build(CB)

    enc = np.asarray(encoder_outputs, dtype=np.float32)
    # [S,B,Hin] -> [B,Hin,S] in fp8 so per-core DMAs are contiguous
    enc_t = np.ascontiguousarray(np.transpose(enc, (1, 2, 0)).astype(F8))
    hid_t = np.asarray(hidden, dtype=np.float32).T.astype(F8)  # [H=k, B]
    W1 = np.asarray(W1, dtype=np.float32)
    w1e8 = (WS * W1[:, :HIN].T).astype(F8)   # [K=HIN, H]
    w1h8 = (WS * W1[:, HIN:].T).astype(F8)   # [K=H, H]
    # w1e packed [ht, p, it, m] = w1e8[it*128+p, ht*128+m]
    w1e_pack = np.ascontiguousarray(
        w1e8.reshape(IT, P, HT, P).transpose(2, 1, 0, 3))
    # w1h packed [p, it, h] = w1h8[it*128+p, h]
    w1h_pack = np.ascontiguousarray(
        w1h8.reshape(IT, P, H).transpose(1, 0, 2))
    b1 = np.ascontiguousarray(np.asarray(b1, dtype=np.float32).reshape(H))
    # w2 packed [p, ht] = W2[ht*128+p], f32 per-partition scalars
    w2_pack = np.ascontiguousarray(
        np.asarray(W2, dtype=np.float32).reshape(HT, P).T)

    in_maps = []
    for c in range(N_CORES):
        sl = slice(c * BL, (c + 1) * BL)
        hid_pack = np.zeros((P, IT, BP), dtype=F8)
        hid_pack[:, :, 0:BL] = hid_t[:, sl].reshape(IT, P, BL).transpose(1, 0, 2)
        enc_pack = np.zeros((HIN, TCOLS), dtype=F8)
        mneg = np.full(TCOLS, -1e30, dtype=np.float32)
        for b in range(BL):
            ix = idx_all[c * BL + b]
            enc_pack[:, b * CB:b * CB + len(ix)] = enc_t[c * BL + b][:, ix]
            mneg[b * CB:b * CB + len(ix)] = 0.0
        in_maps.append({
            "enc": np.ascontiguousarray(enc_pack.reshape(IT, P, TCOLS)),
            "hiddent": hid_pack,
            "maskneg": mneg,
            "w1e": w1e_pack,
            "w1h": w1h_pack,
            "b1": b1,
            "w2": w2_pack,
        })

    res = run_bass_kernel_spmd(nc, in_maps, core_ids=list(range(N_CORES)))
    LAST_RESULT = res
    out = np.zeros((B, S), dtype=np.float32)
    for c in range(N_CORES):
        packed = res.results[c]["out"]
        for b in range(BL):
            gb = c * BL + b
            ix = idx_all[gb]
            out[gb, ix] = packed[b * CB:b * CB + len(ix)]
    return np.ascontiguousarray(out[:, None, :])
